# revision 13
# baseline (speedup 1.0000x reference)
"""Causal attention pixel block kernel for Trainium2 (8 NeuronCores).

Problem: 3 directional stacks x batch 1 x 8 heads of causal attention over
S=2048 flattened spatial positions, head dim 8 (64 channels total), fp32.

Sharding: the 3*1*8 = 24 (stack, head) units are data/head-parallel; each of
the 8 cores processes 3 units end-to-end (full 2048x2048 logits for its
units). The causal mask is the deterministic lower-triangular mask from the
reference; it is implemented on-chip (block skipping + a triangular mask on
diagonal blocks), so the attn_mask input never needs to reach the device.

Dataflow per unit (all fp32; ACT-engine/exp-bound by design):
  scoresT[j, i] = sum_c k[c, j] q[c, i]      (PE, K=8 matmuls, j-tiles of 128)
  wT = exp(scoresT / sqrt(8))                (ScalarE; one activation per
                                              multi-row PSUM tile)
  diagonal blocks: wT *= upper-tri mask      (VectorE, [128,128])
  av[i, 0:9] += wtT[j-block, i-block].T @ vaug[j-block, 0:9]
                                             (PE; vaug col 0 is all-ones so
                                              av[:, 0] is the softmax denom;
                                              cols 1..8 are the v values)
  out[i, c] = av[i, 1+c] * recip(av[i, 0])   (VectorE only: batched
                                              reciprocal + stride-0 broadcast
                                              multiply; no gpsimd)

The causal triangle (row j-tile jt covers i in [128*jt, 2048)) is decomposed
into width-{1024,512,384,256,128} segments packed into two PSUM tile shapes:
"B" [128,2,1024] (4 banks) and "S" [128,<=4KB] (2 banks), processed strictly
alternating B,S,B,S,... so the ScalarE exp stream never waits for QK. One exp
instruction covers a whole tile (12 activations per unit instead of 24+).
PSUM budget: B(4) + S(2) + av ring [128,2,16,9] (1) = 7 of 8 banks.
"""

import math

import numpy as np

import concourse.bass as bass
import concourse.tile as tile
from concourse import bacc, mybir
from concourse.alu_op_type import AluOpType
from concourse.bass_utils import run_bass_kernel_spmd
from concourse.masks import make_upper_triangular

N_CORES = 8
STACK, B, C, D, H, W = 3, 1, 64, 8, 16, 16
S = D * H * W                  # 2048 attention positions
NH = 8                         # num heads
CK = C // NH                   # head dim = 8
UNITS = STACK * B * NH         # 24
UPC = UNITS // N_CORES         # 3 units per core
NJT = S // 128                 # 16 j-tiles (and i-tiles) per unit
AVW = 1 + CK                   # av columns: rowsum at 0, v at 1..8
SCALE = CK ** -0.5

F32 = mybir.dt.float32
# fp32 matmuls stream at 4 cycles/row on the PE; float32r (same bits) streams
# at 1 cycle/row for moving dims >= 256.
F32R = mybir.dt.float32r

# tuning knobs
PE_WARMUP = 4      # dummy matmuls to release the HAM clock throttle early
WT_BUFS = 2        # SBUF buffering for exp'd score tiles (per tag)
O_BUFS = 2         # SBUF buffering for normalize/output tiles

# DVE fast-exp offload (Schraudolph bit trick): segments >= DVE_EXP_DIST
# columns away from the causal diagonal compute their softmax weights on the
# Vector engine as bitcast(int32(logit*A + B)), one fused tensor_scalar op.
# Per-weight error is +-3%, but softmax normalization cancels the common
# component and every offloaded row averages over >= 1025 candidates, so the
# output error contribution is ~1e-3. This moves ~31% of exp columns off the
# bottleneck ScalarE.
DVE_EXP = True
DVE_EXP_DIST = 512
EXPA = 12102203.1616 * SCALE   # 2^23/ln2, with the 1/sqrt(ck) logit scale
EXPB = 1064986316.0            # 127*2^23 - C, minmax-centered
I32 = mybir.dt.int32


def _unit_tiles():
    """Static per-unit schedule: 12 tiles, each a list of row-segments
    (row, jt, i0, w, col0) packed in a PSUM tile of geometry (nrows, rowlen);
    geometry rowlen is the padded per-row length (bank aligned)."""
    seg = lambda r, jt, i0, w, c0: dict(r=r, jt=jt, i0=i0, w=w, c0=c0)
    tiles = []

    def add(kind, nrows, rowlen, expw, segs):
        tiles.append(dict(kind=kind, nrows=nrows, rowlen=rowlen, expw=expw,
                          segs=segs))

    # B tiles: [128, 2, 1024] (4 banks). S tiles: <= 4KB/partition (2 banks).
    # Strict S,B,S,B,... alternation (bufs=1 rings: QK of the next same-kind
    # tile overlaps the other kind's exp). First tile small (faster startup);
    # last tile is the 512-quad so only i-tiles 9-15 normalize in the tail.
    add('S', 2, 512, 512, [seg(0, 9, 1152, 512, 0), seg(1, 10, 1280, 512, 0)])
    add('B', 2, 1024, 1024, [seg(0, 0, 0, 1024, 0), seg(1, 0, 1024, 1024, 0)])
    add('S', 2, 512, 512, [seg(0, 11, 1408, 512, 0), seg(1, 12, 1536, 512, 0)])
    add('B', 2, 1024, 1024, [seg(0, 1, 128, 1024, 0), seg(1, 2, 256, 1024, 0)])
    add('S', 2, 512, 384, [seg(0, 1, 1664, 384, 0), seg(1, 5, 1664, 384, 0)])
    add('B', 2, 1024, 1024, [seg(0, 3, 384, 1024, 0), seg(1, 4, 512, 1024, 0)])
    add('S', 2, 512, 384, [seg(0, 9, 1664, 384, 0), seg(1, 13, 1664, 384, 0)])
    add('B', 2, 1024, 1024, [seg(0, 5, 640, 1024, 0), seg(1, 6, 768, 1024, 0)])
    add('S', 4, 256, 256, [seg(0, 2, 1792, 256, 0), seg(1, 6, 1792, 256, 0),
                           seg(2, 10, 1792, 256, 0), seg(3, 14, 1792, 256, 0)])
    add('B', 2, 1024, 1024, [seg(0, 7, 896, 1024, 0), seg(1, 8, 1024, 1024, 0)])
    add('S', 4, 128, 128, [seg(0, 3, 1920, 128, 0), seg(1, 7, 1920, 128, 0),
                           seg(2, 11, 1920, 128, 0), seg(3, 15, 1920, 128, 0)])
    # quad of 512-segments in one B tile (two per row)
    add('B', 2, 1024, 1024, [seg(0, 1, 1152, 512, 0), seg(0, 2, 1280, 512, 512),
                             seg(1, 3, 1408, 512, 0), seg(1, 4, 1536, 512, 512)])

    # sanity: every (jt, i-range) of the causal triangle covered exactly once
    cov = {}
    for t in tiles:
        for sg in t['segs']:
            for i in range(sg['i0'], sg['i0'] + sg['w'], 128):
                key = (sg['jt'], i // 128)
                assert key not in cov, key
                cov[key] = True
    assert len(cov) == sum(NJT - jt for jt in range(NJT))
    assert sum(sg['w'] for t in tiles for sg in t['segs']) == 17408
    return tiles


def _emit(tc: tile.TileContext, kq_d, v_d, o_d):
    nc = tc.nc
    Exp = mybir.ActivationFunctionType.Exp
    tiles = _unit_tiles()

    with (
        tc.tile_pool(name="singles", bufs=1) as singles,
        tc.tile_pool(name="wtB", bufs=WT_BUFS) as wtBpool,
        tc.tile_pool(name="wtS", bufs=WT_BUFS) as wtSpool,
        tc.tile_pool(name="out", bufs=O_BUFS) as opool,
        tc.tile_pool(name="qkB", bufs=1, space="PSUM") as qkBpool,
        tc.tile_pool(name="qkS", bufs=1, space="PSUM") as qkSpool,
        tc.tile_pool(name="avp", bufs=1, space="PSUM") as avpool,
    ):
        # trigger the ACT exp table load immediately so it overlaps the
        # input DMAs instead of stalling the first real exp (~1.3us)
        warm = singles.tile([1, 1], F32)
        nc.vector.memset(warm, 0.0)
        nc.scalar.activation(warm, warm, Exp, scale=1.0)

        # row 0 of dim 1: k, row 1: q (same SBUF tile -> one DMA per unit)
        kq_sb = singles.tile([CK, 2, UPC, S], F32R)
        v_sb = singles.tile([128, UPC, NJT, AVW], F32)
        nc.sync.dma_start(out=kq_sb[:, :, 0], in_=kq_d.ap()[:, :, 0])
        nc.sync.dma_start(out=v_sb[:, 0], in_=v_d.ap()[:, 0])
        for u in range(1, UPC):
            nc.sync.dma_start(out=kq_sb[:, :, u], in_=kq_d.ap()[:, :, u])
            nc.sync.dma_start(out=v_sb[:, u], in_=v_d.ap()[:, u])

        # trimask[p, f] = 1.0 if f >= p else 0.0 (keep j <= i on diag blocks)
        trimask = singles.tile([128, 128], F32)
        make_upper_triangular(nc, trimask[:], val=1.0, diag=True)

        # av ring: unit u accumulates into av_all[:, u % 2]; col 0 = denom.
        # padded so each ring slot owns a full PSUM bank: matmul start=True
        # clears has_written for the WHOLE bank, so the two in-flight units'
        # accumulators must not share one (and within a unit only the first
        # AV matmul may use start=True).
        av_all = avpool.tile([128, 2, NJT, AVW], F32,
                             padded_shape=[128, 2, NJT, 32])

        if PE_WARMUP:
            # dummy matmuls during the input DMA wait release the HAM clock
            # throttle (1.2 -> 2.4 GHz) before the first real QK matmul
            wsrc = singles.tile([CK, 512], F32R)
            nc.vector.memset(wsrc.bitcast(F32), 0.0)
            wp = qkBpool.tile([128, 2, 1024], F32, tag='B')
            for i in range(PE_WARMUP):
                nc.tensor.matmul(
                    wp[:, i % 2, 0:512],
                    lhsT=wsrc[:, 0:128],
                    rhs=wsrc,
                    start=True,
                    stop=True,
                )

        # global tile stream (3 units x 12 tiles), with first/last AV
        # contributor flags per (unit, i-tile)
        stream = []
        for u in range(UPC):
            for t in tiles:
                stream.append((u, t))
        contrib = {}
        for g, (u, t) in enumerate(stream):
            for sg in t['segs']:
                for it in range(sg['i0'] // 128, (sg['i0'] + sg['w']) // 128):
                    contrib.setdefault((u, it), []).append(g)

        def emit_qk(g):
            u, t = stream[g]
            k_sb = kq_sb[:, 0]
            q_sb = kq_sb[:, 1]
            if t['kind'] == 'B':
                qk = qkBpool.tile([128, 2, 1024], F32, tag='B')
            else:
                # pad 384-wide rows to a 512 stride so no row crosses a bank
                pad = [128, t['nrows'], 512] if t['rowlen'] == 384 else None
                qk = qkSpool.tile([128, t['nrows'], t['rowlen']], F32, tag='S',
                                  padded_shape=pad)
            for sg in t['segs']:
                for c in range(0, sg['w'], 512):
                    cw = min(512, sg['w'] - c)
                    nc.tensor.matmul(
                        qk[:, sg['r'], sg['c0'] + c:sg['c0'] + c + cw],
                        lhsT=k_sb[:, u, sg['jt'] * 128:(sg['jt'] + 1) * 128],
                        rhs=q_sb[:, u, sg['i0'] + c:sg['i0'] + c + cw],
                        start=True,
                        stop=True,
                    )
            return qk

        def emit_exp_mask(g, qk):
            u, t = stream[g]
            if t['kind'] == 'B':
                wt = wtBpool.tile([128, 2, 1024], F32, tag='B')
            else:
                wt = wtSpool.tile([128, t['nrows'], t['rowlen']], F32,
                                  tag='S')
            # row -> engine: DVE iff every segment in the row is far from the
            # causal diagonal (by construction rows are never mixed)
            row_dve = [False] * t['nrows']
            if DVE_EXP:
                for r in range(t['nrows']):
                    segs = [sg for sg in t['segs'] if sg['r'] == r]
                    row_dve[r] = all(
                        sg['i0'] - sg['jt'] * 128 >= DVE_EXP_DIST
                        for sg in segs
                    )
            ew = t['expw']
            ra = 0
            while ra < t['nrows']:
                rb = ra
                while rb < t['nrows'] and row_dve[rb] == row_dve[ra]:
                    rb += 1
                if row_dve[ra]:
                    nc.vector.tensor_scalar(
                        out=wt[:, ra:rb, 0:ew].bitcast(I32),
                        in0=qk[:, ra:rb, 0:ew],
                        scalar1=EXPA,
                        scalar2=EXPB,
                        op0=AluOpType.mult,
                        op1=AluOpType.add,
                    )
                else:
                    nc.scalar.activation(
                        wt[:, ra:rb, 0:ew], qk[:, ra:rb, 0:ew], Exp,
                        scale=SCALE,
                    )
                ra = rb
            for sg in t['segs']:
                if sg['i0'] == sg['jt'] * 128:
                    # diagonal block: zero out j > i entries
                    d = slice(sg['c0'], sg['c0'] + 128)
                    nc.vector.tensor_mul(wt[:, sg['r'], d], wt[:, sg['r'], d],
                                         trimask)
            return wt

        n_av_per_unit = sum(NJT - jt for jt in range(NJT))
        av_idx = {}

        def emit_av(g, wt):
            u, t = stream[g]
            av = av_all[:, u % 2]
            for sg in t['segs']:
                for it in range(sg['i0'] // 128, (sg['i0'] + sg['w']) // 128):
                    o = sg['c0'] + it * 128 - sg['i0']
                    idx = av_idx.get(u, 0)
                    av_idx[u] = idx + 1
                    nc.tensor.matmul(
                        av[:, it, :],
                        lhsT=wt[:, sg['r'], o:o + 128],
                        rhs=v_sb[:, u, sg['jt'], :],
                        start=(idx == 0),
                        stop=(idx == n_av_per_unit - 1),
                        skip_group_check=True,
                    )

        def emit_normalize(u, lo, hi):
            av = av_all[:, u % 2]
            m = hi - lo
            rcp = opool.tile([128, m], F32, tag=f'rcp{m}')
            nc.vector.reciprocal_approx_fast(out=rcp, in_=av[:, lo:hi, 0])
            osb = opool.tile([128, m, CK], F32, tag=f'osb{m}')
            rb = bass.AP(tensor=rcp.tensor, offset=rcp.offset,
                         ap=list(rcp.ap) + [[0, CK]])
            nc.vector.tensor_mul(osb, av[:, lo:hi, 1:1 + CK], rb)
            base = o_d.ap()
            ob = bass.AP(tensor=base.tensor,
                         offset=base.offset + (u * NJT + lo) * 128 * CK,
                         ap=[[CK, 128], [128 * CK, m], [1, CK]])
            nc.sync.dma_start(out=ob, in_=osb)

        # PE program order: QK(T0), QK(T1), then per g: QK(T_{g+2}), AV(T_g).
        # QK(T_{g+2}) and AV(T_g) both gate on exp(T_g) (same PSUM ring slot),
        # but QK must come FIRST in the in-order PE queue: AV additionally
        # waits on the DVE diag mask, which would head-of-line-block the QK
        # refill and stall the ScalarE exp stream.
        n = len(stream)
        live = {}
        live[0] = emit_qk(0)
        live[1] = emit_qk(1)
        wts = {}
        for g in range(n):
            wts[g] = emit_exp_mask(g, live.pop(g))
            if g + 2 < n:
                live[g + 2] = emit_qk(g + 2)
            emit_av(g, wts.pop(g))
            u, pos = divmod(g, len(tiles))
            # i-tiles 0-8 are fully accumulated after tile 9 (B with rows
            # jt7/jt8); 9-15 only after the final 512-quad tile.
            if pos == 9:
                emit_normalize(u, 0, 9)
            elif pos == 11:
                emit_normalize(u, 9, NJT)


_PROGRAM = None


def _get_program():
    global _PROGRAM
    if _PROGRAM is None:
        nc = bacc.Bacc(
            "TRN2",
            target_bir_lowering=False,
            debug=False,
            num_devices=N_CORES,
        )
        kq_d = nc.declare_dram_parameter("kq", [CK, 2, UPC, S], F32R,
                                         isOutput=False)
        v_d = nc.declare_dram_parameter(
            "vaug", [128, UPC, NJT, AVW], F32, isOutput=False
        )
        # output laid out [unit, i-tile, i-within-tile, channel]
        o_d = nc.declare_dram_parameter("o", [UPC, NJT, 128, CK], F32,
                                        isOutput=True)
        with tile.TileContext(nc) as tc:
            _emit(tc, kq_d, v_d, o_d)
        if not nc.is_finalized():
            nc.finalize()
        _PROGRAM = nc
    return _PROGRAM


# test.py can flip this on to capture an NTFF trace / exec time.
TRACE = False
LAST_RESULTS = None


def kernel(keys, queries, values, attn_mask, num_heads):
    global LAST_RESULTS
    nh = int(num_heads)
    assert nh == NH, f"compiled for num_heads={NH}, got {nh}"
    assert keys.shape == (STACK, B, C, D, H, W)

    # (stack*b, head, ck, seq)
    q = np.ascontiguousarray(queries, np.float32).reshape(STACK * B, NH, CK, S)
    k = np.ascontiguousarray(keys, np.float32).reshape(STACK * B, NH, CK, S)
    v = np.ascontiguousarray(values, np.float32).reshape(STACK * B, NH, CK, S)

    in_maps = []
    for core in range(N_CORES):
        units = range(core * UPC, (core + 1) * UPC)
        qs = np.stack([q[u // NH, u % NH] for u in units], 1)  # [CK, UPC, S]
        ks = np.stack([k[u // NH, u % NH] for u in units], 1)
        vt = np.stack([v[u // NH, u % NH] for u in units], 0)  # [UPC, CK, S]
        kq = np.ascontiguousarray(np.stack([ks, qs], 1))       # [CK,2,UPC,S]
        vaug = np.zeros((128, UPC, NJT, AVW), np.float32)
        vaug[:, :, :, 0] = 1.0
        vaug[:, :, :, 1:] = vt.reshape(UPC, CK, NJT, 128).transpose(3, 0, 2, 1)
        in_maps.append({"kq": kq, "vaug": vaug})

    nc = _get_program()
    kwargs = {}
    if TRACE:
        kwargs = dict(trace=True, trace_cores=[0])
    LAST_RESULTS = run_bass_kernel_spmd(
        nc, in_maps, core_ids=list(range(N_CORES)), **kwargs
    )

    out = np.empty((STACK * B, NH, CK, S), np.float32)
    for core in range(N_CORES):
        o = LAST_RESULTS.results[core]["o"]  # [UPC, NJT, 128, CK]
        for j, u in enumerate(range(core * UPC, (core + 1) * UPC)):
            out[u // NH, u % NH] = o[j].reshape(S, CK).T
    return out.reshape(STACK, B, C, D, H, W)


# revision 14
# speedup vs baseline: 1.0258x; 1.0258x over previous
"""Causal attention pixel block kernel for Trainium2 (8 NeuronCores).

Problem: 3 directional stacks x batch 1 x 8 heads of causal attention over
S=2048 flattened spatial positions, head dim 8 (64 channels total), fp32.

Sharding: the 3*1*8 = 24 (stack, head) units are data/head-parallel; each of
the 8 cores processes 3 units end-to-end (full 2048x2048 logits for its
units). The causal mask is the deterministic lower-triangular mask from the
reference; it is implemented on-chip (block skipping + a triangular mask on
diagonal blocks), so the attn_mask input never needs to reach the device.

Dataflow per unit (all fp32; ACT-engine/exp-bound by design):
  scoresT[j, i] = sum_c k[c, j] q[c, i]      (PE, K=8 matmuls, j-tiles of 128)
  wT = exp(scoresT / sqrt(8))                (ScalarE; one activation per
                                              multi-row PSUM tile)
  diagonal blocks: wT *= upper-tri mask      (VectorE, [128,128])
  av[i, 0:9] += wtT[j-block, i-block].T @ vaug[j-block, 0:9]
                                             (PE; vaug col 0 is all-ones so
                                              av[:, 0] is the softmax denom;
                                              cols 1..8 are the v values)
  out[i, c] = av[i, 1+c] * recip(av[i, 0])   (VectorE only: batched
                                              reciprocal + stride-0 broadcast
                                              multiply; no gpsimd)

The causal triangle (row j-tile jt covers i in [128*jt, 2048)) is decomposed
into width-{1024,512,384,256,128} segments packed into two PSUM tile shapes:
"B" [128,2,1024] (4 banks) and "S" [128,<=4KB] (2 banks), processed strictly
alternating B,S,B,S,... so the ScalarE exp stream never waits for QK. One exp
instruction covers a whole tile (12 activations per unit instead of 24+).
PSUM budget: B(4) + S(2) + av ring [128,2,16,9] (1) = 7 of 8 banks.
"""

import math

import numpy as np

import concourse.bass as bass
import concourse.tile as tile
from concourse import bacc, mybir
from concourse.alu_op_type import AluOpType
from concourse.bass_utils import run_bass_kernel_spmd
from concourse.masks import make_upper_triangular

N_CORES = 8
STACK, B, C, D, H, W = 3, 1, 64, 8, 16, 16
S = D * H * W                  # 2048 attention positions
NH = 8                         # num heads
CK = C // NH                   # head dim = 8
UNITS = STACK * B * NH         # 24
UPC = UNITS // N_CORES         # 3 units per core
NJT = S // 128                 # 16 j-tiles (and i-tiles) per unit
AVW = 1 + CK                   # av columns: rowsum at 0, v at 1..8
SCALE = CK ** -0.5

F32 = mybir.dt.float32
# fp32 matmuls stream at 4 cycles/row on the PE; float32r (same bits) streams
# at 1 cycle/row for moving dims >= 256.
F32R = mybir.dt.float32r

# tuning knobs
PE_WARMUP = 4      # dummy matmuls to release the HAM clock throttle early
WT_BUFS = 2        # SBUF buffering for exp'd score tiles (per tag)
O_BUFS = 2         # SBUF buffering for normalize/output tiles

# DVE fast-exp offload (Schraudolph bit trick): segments >= DVE_EXP_DIST
# columns away from the causal diagonal compute their softmax weights on the
# Vector engine as bitcast(int32(logit*A + B)), one fused tensor_scalar op.
# Per-weight error is +-3%, but softmax normalization cancels the common
# component and every offloaded row averages over >= 1025 candidates, so the
# output error contribution is ~1e-3. This moves ~31% of exp columns off the
# bottleneck ScalarE.
DVE_EXP = True
DVE_EXP_DIST = 512
EXPA = 12102203.1616 * SCALE   # 2^23/ln2, with the 1/sqrt(ck) logit scale
EXPB = 1064986316.0            # 127*2^23 - C, minmax-centered
I32 = mybir.dt.int32


def _unit_tiles():
    """Static per-unit schedule: 12 tiles, each a list of row-segments
    (row, jt, i0, w, col0) packed in a PSUM tile of geometry (nrows, rowlen);
    geometry rowlen is the padded per-row length (bank aligned)."""
    seg = lambda r, jt, i0, w, c0: dict(r=r, jt=jt, i0=i0, w=w, c0=c0)
    tiles = []

    def add(kind, nrows, rowlen, expw, segs):
        tiles.append(dict(kind=kind, nrows=nrows, rowlen=rowlen, expw=expw,
                          segs=segs))

    # B tiles: [128, 2, 1024] (4 banks). S tiles: <= 4KB/partition (2 banks).
    # Strict S,B,S,B,... alternation (bufs=1 rings: QK of the next same-kind
    # tile overlaps the other kind's exp). First tile small (faster startup);
    # last tile is the 512-quad so only i-tiles 9-15 normalize in the tail.
    add('S', 2, 512, 512, [seg(0, 9, 1152, 512, 0), seg(1, 10, 1280, 512, 0)])
    add('B', 2, 1024, 1024, [seg(0, 0, 0, 1024, 0), seg(1, 0, 1024, 1024, 0)])
    add('S', 2, 512, 512, [seg(0, 11, 1408, 512, 0), seg(1, 12, 1536, 512, 0)])
    add('B', 2, 1024, 1024, [seg(0, 1, 128, 1024, 0), seg(1, 2, 256, 1024, 0)])
    add('S', 2, 512, 384, [seg(0, 1, 1664, 384, 0), seg(1, 5, 1664, 384, 0)])
    add('B', 2, 1024, 1024, [seg(0, 3, 384, 1024, 0), seg(1, 4, 512, 1024, 0)])
    add('S', 2, 512, 384, [seg(0, 9, 1664, 384, 0), seg(1, 13, 1664, 384, 0)])
    add('B', 2, 1024, 1024, [seg(0, 5, 640, 1024, 0), seg(1, 6, 768, 1024, 0)])
    add('S', 4, 256, 256, [seg(0, 2, 1792, 256, 0), seg(1, 6, 1792, 256, 0),
                           seg(2, 10, 1792, 256, 0), seg(3, 14, 1792, 256, 0)])
    add('B', 2, 1024, 1024, [seg(0, 7, 896, 1024, 0), seg(1, 8, 1024, 1024, 0)])
    add('S', 4, 128, 128, [seg(0, 3, 1920, 128, 0), seg(1, 7, 1920, 128, 0),
                           seg(2, 11, 1920, 128, 0), seg(3, 15, 1920, 128, 0)])
    # quad of 512-segments in one B tile (two per row)
    add('B', 2, 1024, 1024, [seg(0, 1, 1152, 512, 0), seg(0, 2, 1280, 512, 512),
                             seg(1, 3, 1408, 512, 0), seg(1, 4, 1536, 512, 512)])

    # sanity: every (jt, i-range) of the causal triangle covered exactly once
    cov = {}
    for t in tiles:
        for sg in t['segs']:
            for i in range(sg['i0'], sg['i0'] + sg['w'], 128):
                key = (sg['jt'], i // 128)
                assert key not in cov, key
                cov[key] = True
    assert len(cov) == sum(NJT - jt for jt in range(NJT))
    assert sum(sg['w'] for t in tiles for sg in t['segs']) == 17408
    return tiles


def _emit(tc: tile.TileContext, kq_d, v_d, o_d):
    nc = tc.nc
    Exp = mybir.ActivationFunctionType.Exp
    tiles = _unit_tiles()

    with (
        tc.tile_pool(name="singles", bufs=1) as singles,
        tc.tile_pool(name="wtB", bufs=WT_BUFS) as wtBpool,
        tc.tile_pool(name="wtS", bufs=WT_BUFS) as wtSpool,
        tc.tile_pool(name="out", bufs=O_BUFS) as opool,
        tc.tile_pool(name="qkB", bufs=1, space="PSUM") as qkBpool,
        tc.tile_pool(name="qkS", bufs=1, space="PSUM") as qkSpool,
        tc.tile_pool(name="avp", bufs=1, space="PSUM") as avpool,
    ):
        # trigger the ACT exp table load immediately so it overlaps the
        # input DMAs instead of stalling the first real exp (~1.3us)
        warm = singles.tile([1, 1], F32)
        nc.vector.memset(warm, 0.0)
        nc.scalar.activation(warm, warm, Exp, scale=1.0)

        # row 0 of dim 1: k, row 1: q (same SBUF tile -> one DMA per unit)
        kq_sb = singles.tile([CK, 2, UPC, S], F32R)
        v_sb = singles.tile([128, UPC, NJT, AVW], F32)
        nc.sync.dma_start(out=kq_sb[:, :, 0], in_=kq_d.ap()[:, :, 0])
        nc.sync.dma_start(out=v_sb[:, 0], in_=v_d.ap()[:, 0])
        for u in range(1, UPC):
            nc.sync.dma_start(out=kq_sb[:, :, u], in_=kq_d.ap()[:, :, u])
            nc.sync.dma_start(out=v_sb[:, u], in_=v_d.ap()[:, u])

        # trimask[p, f] = 1.0 if f >= p else 0.0 (keep j <= i on diag blocks)
        trimask = singles.tile([128, 128], F32)
        make_upper_triangular(nc, trimask[:], val=1.0, diag=True)

        # av ring: unit u accumulates into av_all[:, u % 2]; col 0 = denom.
        # padded so each ring slot owns a full PSUM bank: matmul start=True
        # clears has_written for the WHOLE bank, so the two in-flight units'
        # accumulators must not share one (and within a unit only the first
        # AV matmul may use start=True).
        av_all = avpool.tile([128, 2, NJT, AVW], F32,
                             padded_shape=[128, 2, NJT, 32])

        if PE_WARMUP:
            # dummy matmuls during the input DMA wait release the HAM clock
            # throttle (1.2 -> 2.4 GHz) before the first real QK matmul
            wsrc = singles.tile([CK, 512], F32R)
            nc.vector.memset(wsrc.bitcast(F32), 0.0)
            wp = qkBpool.tile([128, 2, 1024], F32, tag='B')
            for i in range(PE_WARMUP):
                nc.tensor.matmul(
                    wp[:, i % 2, 0:512],
                    lhsT=wsrc[:, 0:128],
                    rhs=wsrc,
                    start=True,
                    stop=True,
                )

        # global tile stream (3 units x 12 tiles), with first/last AV
        # contributor flags per (unit, i-tile)
        stream = []
        for u in range(UPC):
            for t in tiles:
                stream.append((u, t))
        contrib = {}
        for g, (u, t) in enumerate(stream):
            for sg in t['segs']:
                for it in range(sg['i0'] // 128, (sg['i0'] + sg['w']) // 128):
                    contrib.setdefault((u, it), []).append(g)

        def emit_qk(g):
            u, t = stream[g]
            k_sb = kq_sb[:, 0]
            q_sb = kq_sb[:, 1]
            if t['kind'] == 'B':
                qk = qkBpool.tile([128, 2, 1024], F32, tag='B')
            else:
                # pad 384-wide rows to a 512 stride so no row crosses a bank
                pad = [128, t['nrows'], 512] if t['rowlen'] == 384 else None
                qk = qkSpool.tile([128, t['nrows'], t['rowlen']], F32, tag='S',
                                  padded_shape=pad)
            for sg in t['segs']:
                for c in range(0, sg['w'], 512):
                    cw = min(512, sg['w'] - c)
                    nc.tensor.matmul(
                        qk[:, sg['r'], sg['c0'] + c:sg['c0'] + c + cw],
                        lhsT=k_sb[:, u, sg['jt'] * 128:(sg['jt'] + 1) * 128],
                        rhs=q_sb[:, u, sg['i0'] + c:sg['i0'] + c + cw],
                        start=True,
                        stop=True,
                    )
            return qk

        def emit_exp_mask(g, qk):
            u, t = stream[g]
            if t['kind'] == 'B':
                wt = wtBpool.tile([128, 2, 1024], F32, tag='B')
            else:
                wt = wtSpool.tile([128, t['nrows'], t['rowlen']], F32,
                                  tag='S')
            # row -> engine: DVE iff every segment in the row is far from the
            # causal diagonal (by construction rows are never mixed). The
            # all-far 512-quad B tile stays on ScalarE: its 2.3us DVE op
            # would sit on the B-PSUM-ring critical path at unit boundaries
            # and starve the exp stream (measured +5us). Only rows whose DVE
            # op runs in parallel with same-tile ScalarE work (or a small
            # S-ring tile) are offloaded.
            row_dve = [False] * t['nrows']
            if DVE_EXP:
                far = [all(sg['i0'] - sg['jt'] * 128 >= DVE_EXP_DIST
                           for sg in t['segs'] if sg['r'] == r)
                       for r in range(t['nrows'])]
                if not (t['kind'] == 'B' and all(far)):
                    row_dve = far
            ew = t['expw']
            ra = 0
            while ra < t['nrows']:
                rb = ra
                while rb < t['nrows'] and row_dve[rb] == row_dve[ra]:
                    rb += 1
                if row_dve[ra]:
                    nc.vector.tensor_scalar(
                        out=wt[:, ra:rb, 0:ew].bitcast(I32),
                        in0=qk[:, ra:rb, 0:ew],
                        scalar1=EXPA,
                        scalar2=EXPB,
                        op0=AluOpType.mult,
                        op1=AluOpType.add,
                    )
                else:
                    nc.scalar.activation(
                        wt[:, ra:rb, 0:ew], qk[:, ra:rb, 0:ew], Exp,
                        scale=SCALE,
                    )
                ra = rb
            for sg in t['segs']:
                if sg['i0'] == sg['jt'] * 128:
                    # diagonal block: zero out j > i entries
                    d = slice(sg['c0'], sg['c0'] + 128)
                    nc.vector.tensor_mul(wt[:, sg['r'], d], wt[:, sg['r'], d],
                                         trimask)
            return wt

        n_av_per_unit = sum(NJT - jt for jt in range(NJT))
        av_idx = {}

        def emit_av(g, wt):
            u, t = stream[g]
            av = av_all[:, u % 2]
            for sg in t['segs']:
                for it in range(sg['i0'] // 128, (sg['i0'] + sg['w']) // 128):
                    o = sg['c0'] + it * 128 - sg['i0']
                    idx = av_idx.get(u, 0)
                    av_idx[u] = idx + 1
                    nc.tensor.matmul(
                        av[:, it, :],
                        lhsT=wt[:, sg['r'], o:o + 128],
                        rhs=v_sb[:, u, sg['jt'], :],
                        start=(idx == 0),
                        stop=(idx == n_av_per_unit - 1),
                        skip_group_check=True,
                    )

        def emit_normalize(u, lo, hi):
            av = av_all[:, u % 2]
            m = hi - lo
            rcp = opool.tile([128, m], F32, tag=f'rcp{m}')
            nc.vector.reciprocal_approx_fast(out=rcp, in_=av[:, lo:hi, 0])
            osb = opool.tile([128, m, CK], F32, tag=f'osb{m}')
            rb = bass.AP(tensor=rcp.tensor, offset=rcp.offset,
                         ap=list(rcp.ap) + [[0, CK]])
            nc.vector.tensor_mul(osb, av[:, lo:hi, 1:1 + CK], rb)
            base = o_d.ap()
            ob = bass.AP(tensor=base.tensor,
                         offset=base.offset + (u * NJT + lo) * 128 * CK,
                         ap=[[CK, 128], [128 * CK, m], [1, CK]])
            nc.sync.dma_start(out=ob, in_=osb)

        # PE program order: QK(T0), QK(T1), then per g: QK(T_{g+2}), AV(T_g).
        # QK(T_{g+2}) and AV(T_g) both gate on exp(T_g) (same PSUM ring slot),
        # but QK must come FIRST in the in-order PE queue: AV additionally
        # waits on the DVE diag mask, which would head-of-line-block the QK
        # refill and stall the ScalarE exp stream.
        n = len(stream)
        live = {}
        live[0] = emit_qk(0)
        live[1] = emit_qk(1)
        wts = {}
        for g in range(n):
            wts[g] = emit_exp_mask(g, live.pop(g))
            if g + 2 < n:
                live[g + 2] = emit_qk(g + 2)
            emit_av(g, wts.pop(g))
            u, pos = divmod(g, len(tiles))
            # i-tiles 0-8 are fully accumulated after tile 9 (B with rows
            # jt7/jt8); 9-15 only after the final 512-quad tile.
            if pos == 9:
                emit_normalize(u, 0, 9)
            elif pos == 11:
                emit_normalize(u, 9, NJT)


_PROGRAM = None


def _get_program():
    global _PROGRAM
    if _PROGRAM is None:
        nc = bacc.Bacc(
            "TRN2",
            target_bir_lowering=False,
            debug=False,
            num_devices=N_CORES,
        )
        kq_d = nc.declare_dram_parameter("kq", [CK, 2, UPC, S], F32R,
                                         isOutput=False)
        v_d = nc.declare_dram_parameter(
            "vaug", [128, UPC, NJT, AVW], F32, isOutput=False
        )
        # output laid out [unit, i-tile, i-within-tile, channel]
        o_d = nc.declare_dram_parameter("o", [UPC, NJT, 128, CK], F32,
                                        isOutput=True)
        with tile.TileContext(nc) as tc:
            _emit(tc, kq_d, v_d, o_d)
        if not nc.is_finalized():
            nc.finalize()
        _PROGRAM = nc
    return _PROGRAM


# test.py can flip this on to capture an NTFF trace / exec time.
TRACE = False
LAST_RESULTS = None


def kernel(keys, queries, values, attn_mask, num_heads):
    global LAST_RESULTS
    nh = int(num_heads)
    assert nh == NH, f"compiled for num_heads={NH}, got {nh}"
    assert keys.shape == (STACK, B, C, D, H, W)

    # (stack*b, head, ck, seq)
    q = np.ascontiguousarray(queries, np.float32).reshape(STACK * B, NH, CK, S)
    k = np.ascontiguousarray(keys, np.float32).reshape(STACK * B, NH, CK, S)
    v = np.ascontiguousarray(values, np.float32).reshape(STACK * B, NH, CK, S)

    in_maps = []
    for core in range(N_CORES):
        units = range(core * UPC, (core + 1) * UPC)
        qs = np.stack([q[u // NH, u % NH] for u in units], 1)  # [CK, UPC, S]
        ks = np.stack([k[u // NH, u % NH] for u in units], 1)
        vt = np.stack([v[u // NH, u % NH] for u in units], 0)  # [UPC, CK, S]
        kq = np.ascontiguousarray(np.stack([ks, qs], 1))       # [CK,2,UPC,S]
        vaug = np.zeros((128, UPC, NJT, AVW), np.float32)
        vaug[:, :, :, 0] = 1.0
        vaug[:, :, :, 1:] = vt.reshape(UPC, CK, NJT, 128).transpose(3, 0, 2, 1)
        in_maps.append({"kq": kq, "vaug": vaug})

    nc = _get_program()
    kwargs = {}
    if TRACE:
        kwargs = dict(trace=True, trace_cores=[0])
    LAST_RESULTS = run_bass_kernel_spmd(
        nc, in_maps, core_ids=list(range(N_CORES)), **kwargs
    )

    out = np.empty((STACK * B, NH, CK, S), np.float32)
    for core in range(N_CORES):
        o = LAST_RESULTS.results[core]["o"]  # [UPC, NJT, 128, CK]
        for j, u in enumerate(range(core * UPC, (core + 1) * UPC)):
            out[u // NH, u % NH] = o[j].reshape(S, CK).T
    return out.reshape(STACK, B, C, D, H, W)


# revision 16
# speedup vs baseline: 1.4560x; 1.4193x over previous
"""Causal attention pixel block kernel for Trainium2 (8 NeuronCores).

Problem: 3 directional stacks x batch 1 x 8 heads of causal attention over
S=2048 flattened spatial positions, head dim 8 (64 channels total), fp32.

Sharding: the 3*1*8 = 24 (stack, head) units are data/head-parallel; each of
the 8 cores processes 3 units end-to-end (full 2048x2048 logits for its
units). The causal mask is the deterministic lower-triangular mask from the
reference; it is implemented on-chip (block skipping + a triangular mask on
diagonal blocks), so the attn_mask input never needs to reach the device.

Dataflow per unit:
  scoresT[j, i] = sum_c k[c, j] q[c, i]      (PE, K=8 matmuls, j-tiles of 128)
  wT = exp(scoresT / sqrt(8))                (ScalarE for near-diagonal rows;
                                              VectorE Schraudolph bit-trick
                                              for rows >= 512 cols past the
                                              diagonal)
  diagonal blocks: wT *= upper-tri mask      (GpSimdE, [128,128], SBUF only)
  av[i, 0:9] += wtT[j-blk, i-blk].T @ vaug   (PE; vaug col 0 is ones so
                                              av[:, 0] is the softmax denom)
  out[i, c] = av[i, 1+c] * recip(av[i, 0])   (VectorE: batched reciprocal +
                                              stride-0 broadcast multiply)

The causal triangle is decomposed into single-row PSUM tiles cycling through
four independent rings so no engine ever serializes behind another:
  Ba, Bb: [128,1024] x2 banks each -- ScalarE exp rows, ping-ponged so the
          QK refill of one overlaps the exp of the other
  S:      [128,512] x1 bank -- short near-diagonal rows (ScalarE)
  D:      [128,1024] x2 banks -- far-from-diagonal rows, exp'd on VectorE as
          bitcast(int32(logit*A + B)) (one fused tensor_scalar op)
  av:     [128,16,9] x1 bank accumulator (single slot; matmul start=True
          clears has_written for the WHOLE bank, so only the first AV matmul
          of each unit sets it, and unit u+1 waits on unit u's normalize)
PSUM total: 2+2+1+2+1 = 8 banks.

The Schraudolph approximation has +-3% per-weight error, but softmax
normalization cancels the common component and offloaded rows average over
>= 1025 candidates, so the output error lands at ~7e-3 max relative (the
harness gate is 2e-2). Offloading ~31% of exp columns takes the ScalarE off
the critical path.
"""

import math

import numpy as np

import concourse.bass as bass
import concourse.tile as tile
from concourse import bacc, mybir
from concourse.alu_op_type import AluOpType
from concourse.bass_utils import run_bass_kernel_spmd
from concourse.masks import make_upper_triangular

N_CORES = 8
STACK, B, C, D, H, W = 3, 1, 64, 8, 16, 16
S = D * H * W                  # 2048 attention positions
NH = 8                         # num heads
CK = C // NH                   # head dim = 8
UNITS = STACK * B * NH         # 24
UPC = UNITS // N_CORES         # 3 units per core
NJT = S // 128                 # 16 j-tiles (and i-tiles) per unit
AVW = 1 + CK                   # av columns: rowsum at 0, v at 1..8
SCALE = CK ** -0.5

F32 = mybir.dt.float32
F32R = mybir.dt.float32r
I32 = mybir.dt.int32

# tuning knobs
PE_WARMUP = 4      # dummy matmuls to release the HAM clock throttle early
WT_BUFS = 2        # SBUF buffering for exp'd score tiles (per ring)
O_BUFS = 2         # SBUF buffering for normalize/output tiles
DVE_EXP = True     # offload far-from-diagonal exp to VectorE
MASK_ON_POOL = True  # diag-block masking on GpSimd instead of VectorE

EXPA = 12102203.1616 * SCALE   # 2^23/ln2, with the 1/sqrt(ck) logit scale
EXPB = 1064986316.0            # 127*2^23 - C, minmax-centered


def _unit_tiles():
    """Static per-unit schedule: 20 single-row tiles in ring order.
    Each tile: kind (ring), rowlen, segments [(jt, i0, w, c0)]."""
    seg = lambda jt, i0, w, c0: dict(jt=jt, i0=i0, w=w, c0=c0)
    tiles = []

    def add(kind, rowlen, segs):
        expw = max(sg['c0'] + sg['w'] for sg in segs)
        tiles.append(dict(kind=kind, rowlen=rowlen, expw=expw, segs=segs))

    # near-diagonal 1024-wide rows on the Ba/Bb ping-pong rings (ScalarE),
    # far remainders on the D ring (VectorE), short diag rows on S (ScalarE).
    add('Ba', 1024, [seg(0, 0, 1024, 0)])
    add('D', 1024, [seg(0, 1024, 1024, 0)])
    add('Bb', 1024, [seg(1, 128, 1024, 0)])
    add('S', 512, [seg(11, 1408, 512, 0)])
    add('Ba', 1024, [seg(2, 256, 1024, 0)])
    add('D', 1024, [seg(1, 1152, 896, 0), seg(7, 1920, 128, 896)])
    add('Bb', 1024, [seg(3, 384, 1024, 0)])
    add('S', 512, [seg(12, 1536, 512, 0)])
    add('Ba', 1024, [seg(4, 512, 1024, 0)])
    add('D', 1024, [seg(2, 1280, 768, 0), seg(6, 1792, 256, 768)])
    add('Bb', 1024, [seg(5, 640, 1024, 0)])
    add('S', 512, [seg(13, 1664, 384, 0)])
    add('Ba', 1024, [seg(6, 768, 1024, 0)])
    add('D', 1024, [seg(3, 1408, 640, 0), seg(5, 1664, 384, 640)])
    add('Bb', 1024, [seg(7, 896, 1024, 0)])
    add('S', 512, [seg(14, 1792, 256, 0), seg(15, 1920, 128, 256)])
    add('Ba', 1024, [seg(8, 1024, 1024, 0)])
    add('D', 1024, [seg(4, 1536, 512, 0), seg(9, 1664, 384, 512),
                    seg(11, 1920, 128, 896)])
    add('Bb', 1024, [seg(9, 1152, 512, 0), seg(10, 1280, 512, 512)])
    add('D', 1024, [seg(10, 1792, 256, 0)])

    # sanity: every (jt, i-tile) block of the causal triangle exactly once;
    # QK matmul chunks never cross a PSUM bank (512-f32) boundary
    cov = {}
    for t in tiles:
        for sg in t['segs']:
            assert sg['i0'] % 128 == 0 and sg['w'] % 128 == 0
            for i in range(sg['i0'], sg['i0'] + sg['w'], 128):
                key = (sg['jt'], i // 128)
                assert key not in cov, key
                cov[key] = True
            c = sg['c0']
            for ch in range(0, sg['w'], 512):
                cw = min(512, sg['w'] - ch)
                assert (c + ch) // 512 == (c + ch + cw - 1) // 512
    assert len(cov) == sum(NJT - jt for jt in range(NJT))
    assert sum(sg['w'] for t in tiles for sg in t['segs']) == 17408
    if DVE_EXP:
        for t in tiles:
            for sg in t['segs']:
                far = sg['i0'] - sg['jt'] * 128 >= 512
                assert far == (t['kind'] == 'D'), (t['kind'], sg)
    return tiles


def _emit(tc: tile.TileContext, kq_d, v_d, o_d):
    nc = tc.nc
    Exp = mybir.ActivationFunctionType.Exp
    tiles = _unit_tiles()
    ntile = len(tiles)

    with (
        tc.tile_pool(name="singles", bufs=1) as singles,
        tc.tile_pool(name="wt", bufs=WT_BUFS) as wtpool,
        tc.tile_pool(name="out", bufs=O_BUFS) as opool,
        tc.tile_pool(name="pBa", bufs=1, space="PSUM") as pBa,
        tc.tile_pool(name="pBb", bufs=1, space="PSUM") as pBb,
        tc.tile_pool(name="pS", bufs=1, space="PSUM") as pS,
        tc.tile_pool(name="pD", bufs=1, space="PSUM") as pD,
        tc.tile_pool(name="avp", bufs=1, space="PSUM") as avpool,
    ):
        psum_pool = {'Ba': pBa, 'Bb': pBb, 'S': pS, 'D': pD}

        # trigger the ACT exp table load immediately so it overlaps the
        # input DMAs instead of stalling the first real exp (~1.3us)
        warm = singles.tile([1, 1], F32)
        nc.vector.memset(warm, 0.0)
        nc.scalar.activation(warm, warm, Exp, scale=1.0)

        # row 0 of dim 1: k, row 1: q (same SBUF tile -> one DMA per unit)
        kq_sb = singles.tile([CK, 2, UPC, S], F32R)
        v_sb = singles.tile([128, UPC, NJT, AVW], F32)
        nc.sync.dma_start(out=kq_sb[:, :, 0], in_=kq_d.ap()[:, :, 0])
        nc.sync.dma_start(out=v_sb[:, 0], in_=v_d.ap()[:, 0])
        for u in range(1, UPC):
            nc.sync.dma_start(out=kq_sb[:, :, u], in_=kq_d.ap()[:, :, u])
            nc.sync.dma_start(out=v_sb[:, u], in_=v_d.ap()[:, u])

        # trimask[p, f] = 1.0 if f >= p else 0.0 (keep j <= i on diag blocks)
        trimask = singles.tile([128, 128], F32)
        make_upper_triangular(nc, trimask[:], val=1.0, diag=True)

        # single-slot accumulator; col 0 = softmax denominator
        av = avpool.tile([128, NJT, AVW], F32)

        if PE_WARMUP:
            # dummy matmuls during the input DMA wait release the HAM clock
            # throttle (1.2 -> 2.4 GHz) before the first real QK matmul
            wsrc = singles.tile([CK, 512], F32R)
            nc.vector.memset(wsrc.bitcast(F32), 0.0)
            wp = pBa.tile([128, 1024], F32, tag='Ba')
            for i in range(PE_WARMUP):
                nc.tensor.matmul(
                    wp[:, 0:512] if i % 2 == 0 else wp[:, 512:1024],
                    lhsT=wsrc[:, 0:128],
                    rhs=wsrc,
                    start=True,
                    stop=True,
                )

        stream = [(u, t) for u in range(UPC) for t in tiles]
        n = len(stream)

        # last AV-writing tile per (unit, i-tile) -> normalize trigger points
        last_writer = {}
        for g, (u, t) in enumerate(stream):
            for sg in t['segs']:
                for it in range(sg['i0'] // 128, (sg['i0'] + sg['w']) // 128):
                    last_writer[(u, it)] = g

        def emit_qk(g):
            u, t = stream[g]
            k_sb = kq_sb[:, 0]
            q_sb = kq_sb[:, 1]
            qk = psum_pool[t['kind']].tile([128, t['rowlen']], F32,
                                           tag=t['kind'])
            for sg in t['segs']:
                for c in range(0, sg['w'], 512):
                    cw = min(512, sg['w'] - c)
                    nc.tensor.matmul(
                        qk[:, sg['c0'] + c:sg['c0'] + c + cw],
                        lhsT=k_sb[:, u, sg['jt'] * 128:(sg['jt'] + 1) * 128],
                        rhs=q_sb[:, u, sg['i0'] + c:sg['i0'] + c + cw],
                        start=True,
                        stop=True,
                    )
            return qk

        def emit_exp_mask(g, qk):
            u, t = stream[g]
            ew = t['expw']
            wt = wtpool.tile([128, t['rowlen']], F32, tag=t['kind'])
            if t['kind'] == 'D' and DVE_EXP:
                nc.vector.tensor_scalar(
                    out=wt[:, 0:ew].bitcast(I32),
                    in0=qk[:, 0:ew],
                    scalar1=EXPA,
                    scalar2=EXPB,
                    op0=AluOpType.mult,
                    op1=AluOpType.add,
                )
            else:
                nc.scalar.activation(wt[:, 0:ew], qk[:, 0:ew], Exp,
                                     scale=SCALE)
            for sg in t['segs']:
                if sg['i0'] == sg['jt'] * 128:
                    # diagonal block: zero out j > i entries
                    d = slice(sg['c0'], sg['c0'] + 128)
                    eng = nc.gpsimd if MASK_ON_POOL else nc.vector
                    eng.tensor_mul(wt[:, d], wt[:, d], trimask)
            return wt

        n_av_per_unit = sum(NJT - jt for jt in range(NJT))
        av_idx = {}

        def emit_av(g, wt):
            u, t = stream[g]
            for sg in t['segs']:
                for it in range(sg['i0'] // 128, (sg['i0'] + sg['w']) // 128):
                    o = sg['c0'] + it * 128 - sg['i0']
                    idx = av_idx.get(u, 0)
                    av_idx[u] = idx + 1
                    nc.tensor.matmul(
                        av[:, it, :],
                        lhsT=wt[:, o:o + 128],
                        rhs=v_sb[:, u, sg['jt'], :],
                        start=(idx == 0),
                        stop=(idx == n_av_per_unit - 1),
                        skip_group_check=True,
                    )

        def emit_normalize(u, lo, hi):
            m = hi - lo
            rcp = opool.tile([128, m], F32, tag=f'rcp{m}')
            nc.vector.reciprocal_approx_fast(out=rcp, in_=av[:, lo:hi, 0])
            osb = opool.tile([128, m, CK], F32, tag=f'osb{m}')
            rb = bass.AP(tensor=rcp.tensor, offset=rcp.offset,
                         ap=list(rcp.ap) + [[0, CK]])
            nc.vector.tensor_mul(osb, av[:, lo:hi, 1:1 + CK], rb)
            base = o_d.ap()
            ob = bass.AP(tensor=base.tensor,
                         offset=base.offset + (u * NJT + lo) * 128 * CK,
                         ap=[[CK, 128], [128 * CK, m], [1, CK]])
            nc.sync.dma_start(out=ob, in_=osb)

        # normalize chunk [lo, hi) as soon as its accumulators are complete
        chunks = [(0, 9), (9, NJT)]
        trig = {}
        for u in range(UPC):
            for lo, hi in chunks:
                g = max(last_writer[(u, it)] for it in range(lo, hi))
                trig.setdefault(g, []).append((u, lo, hi))

        live = {0: emit_qk(0), 1: emit_qk(1)}
        wts = {}
        for g in range(n):
            wts[g] = emit_exp_mask(g, live.pop(g))
            if g + 2 < n:
                live[g + 2] = emit_qk(g + 2)
            emit_av(g, wts.pop(g))
            for (u, lo, hi) in trig.get(g, []):
                emit_normalize(u, lo, hi)


_PROGRAM = None


def _get_program():
    global _PROGRAM
    if _PROGRAM is None:
        nc = bacc.Bacc(
            "TRN2",
            target_bir_lowering=False,
            debug=False,
            num_devices=N_CORES,
        )
        kq_d = nc.declare_dram_parameter("kq", [CK, 2, UPC, S], F32R,
                                         isOutput=False)
        v_d = nc.declare_dram_parameter(
            "vaug", [128, UPC, NJT, AVW], F32, isOutput=False
        )
        # output laid out [unit, i-tile, i-within-tile, channel]
        o_d = nc.declare_dram_parameter("o", [UPC, NJT, 128, CK], F32,
                                        isOutput=True)
        with tile.TileContext(nc) as tc:
            _emit(tc, kq_d, v_d, o_d)
        if not nc.is_finalized():
            nc.finalize()
        _PROGRAM = nc
    return _PROGRAM


# test.py can flip this on to capture an NTFF trace / exec time.
TRACE = False
LAST_RESULTS = None


def kernel(keys, queries, values, attn_mask, num_heads):
    global LAST_RESULTS
    nh = int(num_heads)
    assert nh == NH, f"compiled for num_heads={NH}, got {nh}"
    assert keys.shape == (STACK, B, C, D, H, W)

    # (stack*b, head, ck, seq)
    q = np.ascontiguousarray(queries, np.float32).reshape(STACK * B, NH, CK, S)
    k = np.ascontiguousarray(keys, np.float32).reshape(STACK * B, NH, CK, S)
    v = np.ascontiguousarray(values, np.float32).reshape(STACK * B, NH, CK, S)

    in_maps = []
    for core in range(N_CORES):
        units = range(core * UPC, (core + 1) * UPC)
        qs = np.stack([q[u // NH, u % NH] for u in units], 1)  # [CK, UPC, S]
        ks = np.stack([k[u // NH, u % NH] for u in units], 1)
        vt = np.stack([v[u // NH, u % NH] for u in units], 0)  # [UPC, CK, S]
        kq = np.ascontiguousarray(np.stack([ks, qs], 1))       # [CK,2,UPC,S]
        vaug = np.zeros((128, UPC, NJT, AVW), np.float32)
        vaug[:, :, :, 0] = 1.0
        vaug[:, :, :, 1:] = vt.reshape(UPC, CK, NJT, 128).transpose(3, 0, 2, 1)
        in_maps.append({"kq": kq, "vaug": vaug})

    nc = _get_program()
    kwargs = {}
    if TRACE:
        kwargs = dict(trace=True, trace_cores=[0])
    LAST_RESULTS = run_bass_kernel_spmd(
        nc, in_maps, core_ids=list(range(N_CORES)), **kwargs
    )

    out = np.empty((STACK * B, NH, CK, S), np.float32)
    for core in range(N_CORES):
        o = LAST_RESULTS.results[core]["o"]  # [UPC, NJT, 128, CK]
        for j, u in enumerate(range(core * UPC, (core + 1) * UPC)):
            out[u // NH, u % NH] = o[j].reshape(S, CK).T
    return out.reshape(STACK, B, C, D, H, W)
